# revision 10
# baseline (speedup 1.0000x reference)
"""Trainium2 Bass kernel for nn_Attention_84026740179215.

Multi-head attention: x[8,1024,768] -> qkv -> per-head softmax(QK^T/sqrt(d))V -> proj.
Sharding: pure data parallel, one batch element per NeuronCore (B=8 = 8 cores).

Per-core layout strategy (N=1024 tokens, C=768, H=12 heads, D=64):
  - Host ships x[b].T so the contraction dim is on partitions everywhere.
  - q^T,k^T computed as [c', n] (lhsT = W_qkv native layout, rhs = x^T).
  - V computed in natural [m, d] layout, augmented with a ones column per
    head so the PV matmul also produces the softmax denominator Z for free.
  - S^T[m, n] per head (K=64), exp via ACT (scale=1/8 fused) PSUM->SBUF,
    PV accumulates out_aug^T[65, n]; row 64 is Z. Normalization by 1/Z via
    reciprocal_approx_fast + gpsimd partition_broadcast, written straight
    into the projection's lhsT layout [c', n]. No max-subtraction in the
    softmax: scores are ~N(0,1) here, exp is safe in fp32.
  - All matmul operands are float32r (TF32-like, 1 cycle/row, ~1e-4 rel err).
"""

import numpy as np

import concourse.bacc as bacc
import concourse.bass as bass
import concourse.mybir as mybir
import concourse.tile as tile
from concourse import bass_utils

N_CORES = 8
N = 1024          # tokens per batch element
C = 768           # model dim
H = 12            # heads
D = 64            # head dim
KT = C // 128     # 6 k-tiles of the contraction dim
NCH = N // 128    # 8 chunks of the token dim (query side)
MT = N // 128     # 8 tiles of the token dim (key/value side)

DEBUG_TAPS = False
F32 = mybir.dt.float32
F32R = mybir.dt.float32r
AF = mybir.ActivationFunctionType


def _build():
    nc = bacc.Bacc("TRN2", target_bir_lowering=False, debug=False,
                   num_devices=N_CORES)

    xT = nc.dram_tensor("xT", [C, N], F32R, kind="ExternalInput")
    w_qkv = nc.dram_tensor("w_qkv", [C, 3 * C], F32R, kind="ExternalInput")
    w_proj = nc.dram_tensor("w_proj", [C, C], F32R, kind="ExternalInput")
    b_qk = nc.dram_tensor("b_qk", [2 * KT, 128, 1], F32, kind="ExternalInput")
    b_v = nc.dram_tensor("b_v", [128, C], F32, kind="ExternalInput")
    b_o = nc.dram_tensor("b_o", [128, C], F32, kind="ExternalInput")
    ones12 = nc.dram_tensor("ones12", [128, H], F32R, kind="ExternalInput")
    y = nc.dram_tensor("y", [N, C], F32, kind="ExternalOutput")
    dbg = {}
    if DEBUG_TAPS:
        dbg["oa0"] = nc.dram_tensor("oa0", [D + 1, N], F32, kind="ExternalOutput")
        dbg["rz0"] = nc.dram_tensor("rz0", [1, N], F32, kind="ExternalOutput")
        dbg["rzb0"] = nc.dram_tensor("rzb0", [D, N], F32, kind="ExternalOutput")
        dbg["onorm0"] = nc.dram_tensor("onorm0", [128, N], F32, kind="ExternalOutput")
        dbg["pt00"] = nc.dram_tensor("pt00", [128, N], F32, kind="ExternalOutput")
        dbg["qk0"] = nc.dram_tensor("qk0", [128, N], F32, kind="ExternalOutput")
        dbg["kk0"] = nc.dram_tensor("kk0", [128, N], F32, kind="ExternalOutput")
        dbg["va0"] = nc.dram_tensor("va0", [128, H * (D + 1)], F32, kind="ExternalOutput")

    with tile.TileContext(nc) as tc:
        _body(nc, tc, xT, w_qkv, w_proj, b_qk, b_v, b_o, ones12, y, dbg)
    nc.compile()
    return nc


def _body(nc, tc, xT, w_qkv, w_proj, b_qk, b_v, b_o, ones12, y, dbg={}):
    import contextlib
    ctx = contextlib.ExitStack()
    with ctx:
        big = ctx.enter_context(tc.tile_pool(name="big", bufs=9))
        qk_pool = ctx.enter_context(tc.tile_pool(name="qk", bufs=1))
        vaug_pool = ctx.enter_context(tc.tile_pool(name="vaug", bufs=1))
        onorm_pool = ctx.enter_context(tc.tile_pool(name="onorm", bufs=1))
        wqk_pool = ctx.enter_context(tc.tile_pool(name="wqk", bufs=8))
        wv_pool = ctx.enter_context(tc.tile_pool(name="wv", bufs=1))
        bias_pool = ctx.enter_context(tc.tile_pool(name="bias", bufs=1))
        ysb_pool = ctx.enter_context(tc.tile_pool(name="ysb", bufs=2))
        rz_pool = ctx.enter_context(tc.tile_pool(name="rz", bufs=2))
        rzb_pool = ctx.enter_context(tc.tile_pool(name="rzb", bufs=2))
        psum = ctx.enter_context(tc.tile_pool(name="ps", bufs=4, space="PSUM"))

        # ---- stage resident inputs ----
        xt_sb = []
        for kt in range(KT):
            t = big.tile([128, N], F32R, tag="big")
            nc.sync.dma_start(t[:], xT.ap()[kt * 128:(kt + 1) * 128, :])
            xt_sb.append(t)

        wv_sb = []
        for kt in range(KT):
            t = wv_pool.tile([128, C], F32R, tag=f"w2_{kt}", name=f"wv{kt}")
            nc.sync.dma_start(
                t[:], w_qkv.ap()[kt * 128:(kt + 1) * 128, 2 * C:3 * C])
            wv_sb.append(t)

        bqk_sb = []
        for cc in range(2 * KT):
            t = bias_pool.tile([128, 1], F32, tag=f"bqk{cc}", name=f"bqk{cc}")
            nc.sync.dma_start(t[:], b_qk.ap()[cc])
            bqk_sb.append(t)
        bv_sb = bias_pool.tile([128, C], F32, tag="bv")
        nc.sync.dma_start(bv_sb[:], b_v.ap())
        bo_sb = bias_pool.tile([128, C], F32, tag="bo")
        nc.sync.dma_start(bo_sb[:], b_o.ap())

        # ---- phase 1a: q^T / k^T chunks [c', n], interleaved q,k so heads
        # can start as soon as their chunk pair lands ----
        qk_sb = {}
        for j in range(KT):
            for cc in (j, KT + j):          # q chunk j, then k chunk j
                pc = psum.tile([128, N], F32, tag="ps")
                for kt in range(KT):
                    wt = wqk_pool.tile([128, 128], F32R, tag="wqk")
                    nc.sync.dma_start(
                        wt[:],
                        w_qkv.ap()[kt * 128:(kt + 1) * 128,
                                   cc * 128:(cc + 1) * 128])
                    for half in range(2):
                        s = slice(half * 512, (half + 1) * 512)
                        nc.tensor.matmul(pc[:, s], wt[:], xt_sb[kt][:, s],
                                         start=(kt == 0), stop=(kt == KT - 1))
                t = qk_pool.tile([128, N], F32R, tag=f"qk{cc}", name=f"qkc{cc}")
                nc.vector.tensor_scalar_add(t[:], pc[:], bqk_sb[cc][:])
                qk_sb[cc] = t

        # ---- phase 1b: V natural [m, d] with ones column per head ----
        # vaug[mt] cols: head h occupies [h*65, h*65+64) = v, col h*65+64 = 1.0
        vaug_sb = []
        for mt in range(MT):
            vc = psum.tile([128, C], F32, tag="ps")
            for kt in range(KT):
                nc.tensor.matmul(vc[:, 0:512],
                                 xt_sb[kt][:, mt * 128:(mt + 1) * 128],
                                 wv_sb[kt][:, 0:512],
                                 start=(kt == 0), stop=(kt == KT - 1))
                nc.tensor.matmul(vc[:, 512:768],
                                 xt_sb[kt][:, mt * 128:(mt + 1) * 128],
                                 wv_sb[kt][:, 512:768],
                                 start=(kt == 0), stop=(kt == KT - 1))
            va = vaug_pool.tile([128, H * (D + 1)], F32R, tag=f"vaug{mt}", name=f"vaug{mt}")
            va_h = va[:].rearrange("p (h s) -> p h s", h=H)
            nc.sync.dma_start(va_h[:, :, D], ones12.ap())
            nc.vector.tensor_add(
                va_h[:, :, 0:D],
                vc[:].rearrange("p (h s) -> p h s", h=H),
                bv_sb[:].rearrange("p (h s) -> p h s", h=H))
            vaug_sb.append(va)

        # ---- stage w_proj, reusing the wv slots (same tags) ----
        wp_sb = []
        for kt in range(KT):
            t = wv_pool.tile([128, C], F32R, tag=f"w2_{kt}", name=f"wp{kt}")
            nc.sync.dma_start(t[:], w_proj.ap()[kt * 128:(kt + 1) * 128, :])
            wp_sb.append(t)

        # ---- phase 2: attention per head ----
        onorm_sb = [onorm_pool.tile([128, N], F32R, tag=f"onorm{i}",
                                    name=f"onorm{i}")
                    for i in range(KT)]
        for h in range(H):
            qt = qk_sb[h // 2][(h % 2) * D:(h % 2) * D + D, :]
            kt_ap = qk_sb[KT + h // 2][(h % 2) * D:(h % 2) * D + D, :]

            oa = psum.tile([128, N], F32, tag="ps")
            pts = []
            LOOKAHEAD = 2

            def do_st(mc):
                st = psum.tile([128, N], F32, tag="ps")
                for half in range(2):
                    s = slice(half * 512, (half + 1) * 512)
                    nc.tensor.matmul(st[:, s],
                                     kt_ap[:, mc * 128:(mc + 1) * 128],
                                     qt[:, s], start=True, stop=True)
                pt = big.tile([128, N], F32R, tag="big")
                nc.scalar.activation(pt[:], st[:], AF.Exp, scale=float(D) ** -0.5)
                pts.append(pt)

            def do_pv(mc):
                for half in range(2):
                    s = slice(half * 512, (half + 1) * 512)
                    nc.tensor.matmul(
                        oa[0:D + 1, s],
                        vaug_sb[mc][:, h * (D + 1):(h + 1) * (D + 1)],
                        pts[mc][:, s],
                        start=(mc == 0), stop=(mc == MT - 1))

            for mc in range(MT):
                do_st(mc)
                if mc >= LOOKAHEAD:
                    do_pv(mc - LOOKAHEAD)
            for mc in range(MT - LOOKAHEAD, MT):
                do_pv(mc)

            if dbg and h == 0:
                oadbg = rzb_pool.tile([D + 1, N], F32, tag="oadbg", name="oadbg")
                nc.vector.tensor_copy(oadbg[:], oa[0:D + 1, :])
                nc.sync.dma_start(dbg["oa0"].ap(), oadbg[:])
                nc.sync.dma_start(dbg["pt00"].ap(), pts[0][:].bitcast(F32))
                nc.sync.dma_start(dbg["qk0"].ap(), qk_sb[0][:].bitcast(F32))
                nc.sync.dma_start(dbg["kk0"].ap(), qk_sb[KT][:].bitcast(F32))
                nc.sync.dma_start(dbg["va0"].ap(), vaug_sb[0][:].bitcast(F32))
            # normalization: row D of oa is Z_h[n]
            zrow = rz_pool.tile([1, N], F32, tag="zrow")
            nc.vector.tensor_copy(zrow[:], oa[D:D + 1, :])
            rz = rz_pool.tile([1, N], F32, tag="rz")
            nc.vector.reciprocal_approx_fast(rz[:], zrow[:])
            rzb = rzb_pool.tile([D, N], F32, tag="rzb")
            nc.gpsimd.partition_broadcast(rzb[:], rz[:])
            nc.vector.tensor_mul(
                onorm_sb[h // 2][(h % 2) * D:(h % 2) * D + D, :],
                oa[0:D, :], rzb[:])
            if dbg and h == 0:
                nc.sync.dma_start(dbg["rz0"].ap(), rz[:])
                nc.sync.dma_start(dbg["rzb0"].ap(), rzb[:])
            if dbg and h == 1:
                nc.sync.dma_start(dbg["onorm0"].ap(), onorm_sb[0][:].bitcast(F32))

        # ---- phase 3: projection y[n, c] ----
        for nch in range(NCH):
            yp = psum.tile([128, C], F32, tag="ps")
            for kt in range(KT):
                nc.tensor.matmul(yp[:, 0:512],
                                 onorm_sb[kt][:, nch * 128:(nch + 1) * 128],
                                 wp_sb[kt][:, 0:512],
                                 start=(kt == 0), stop=(kt == KT - 1))
                nc.tensor.matmul(yp[:, 512:768],
                                 onorm_sb[kt][:, nch * 128:(nch + 1) * 128],
                                 wp_sb[kt][:, 512:768],
                                 start=(kt == 0), stop=(kt == KT - 1))
            ys = ysb_pool.tile([128, C], F32, tag="ysb")
            nc.vector.tensor_add(ys[:], yp[:], bo_sb[:])
            nc.sync.dma_start(y.ap()[nch * 128:(nch + 1) * 128, :], ys[:])


_NC_CACHE = None


def _get_nc():
    global _NC_CACHE
    if _NC_CACHE is None:
        _NC_CACHE = _build()
    return _NC_CACHE


def make_in_maps(x, w_qkv, b_qkv, w_proj, b_proj):
    x = np.asarray(x, np.float32)
    w_qkv = np.ascontiguousarray(np.asarray(w_qkv, np.float32))
    b_qkv = np.asarray(b_qkv, np.float32)
    w_proj = np.ascontiguousarray(np.asarray(w_proj, np.float32))
    b_proj = np.asarray(b_proj, np.float32)

    b_qk = np.ascontiguousarray(b_qkv[:2 * C].reshape(2 * KT, 128, 1))
    b_v = np.ascontiguousarray(
        np.broadcast_to(b_qkv[2 * C:], (128, C)).astype(np.float32))
    b_o = np.ascontiguousarray(
        np.broadcast_to(b_proj, (128, C)).astype(np.float32))
    ones = np.ones((128, H), np.float32)

    in_maps = []
    for c in range(N_CORES):
        in_maps.append({
            "xT": np.ascontiguousarray(x[c].T),
            "w_qkv": w_qkv,
            "w_proj": w_proj,
            "b_qk": b_qk,
            "b_v": b_v,
            "b_o": b_o,
            "ones12": ones,
        })
    return in_maps


def kernel(x, w_qkv, b_qkv, w_proj, b_proj):
    nc = _get_nc()
    in_maps = make_in_maps(x, w_qkv, b_qkv, w_proj, b_proj)
    res = bass_utils.run_bass_kernel_spmd(nc, in_maps, list(range(N_CORES)))
    out = np.stack([res.results[c]["y"] for c in range(N_CORES)], axis=0)
    return out.astype(np.float32)


# revision 22
# speedup vs baseline: 160.2478x; 160.2478x over previous
"""Trainium2 Bass kernel for nn_Attention_84026740179215.

Multi-head attention: x[8,1024,768] -> qkv -> per-head softmax(QK^T/sqrt(d))V -> proj.
Sharding: pure data parallel, one batch element per NeuronCore (B=8 = 8 cores).

Per-core layout strategy (N=1024 tokens, C=768, H=12 heads, D=64):
  - Host ships x[b].T so the contraction dim is on partitions everywhere.
  - q^T,k^T computed as [c', n] (lhsT = W_qkv native layout, rhs = x^T).
  - V computed in natural [m, d] layout, augmented with a ones column per
    head so the PV matmul also produces the softmax denominator Z for free.
  - S^T[m, n] per head (K=64), exp via ACT (scale=1/8 fused) PSUM->SBUF,
    PV accumulates out_aug^T[65, n]; row 64 is Z. Normalization by 1/Z via
    reciprocal_approx_fast + gpsimd partition_broadcast, written straight
    into the projection's lhsT layout [c', n]. No max-subtraction in the
    softmax: scores are ~N(0,1) here, exp is safe in fp32.
  - All matmul operands are float32r (TF32-like, 1 cycle/row, ~1e-4 rel err).
"""

import numpy as np

import concourse.bacc as bacc
import concourse.bass as bass
import concourse.mybir as mybir
import concourse.tile as tile
from concourse import bass_utils

N_CORES = 8
N = 1024          # tokens per batch element
C = 768           # model dim
H = 12            # heads
D = 64            # head dim
KT = C // 128     # 6 k-tiles of the contraction dim
NCH = N // 128    # 8 chunks of the token dim (query side)
MT = N // 128     # 8 tiles of the token dim (key/value side)

DEBUG_TAPS = False
BENCH_ITERS = 0      # >0: wrap the body in a For_i loop (timing harness only)
F32 = mybir.dt.float32
F32R = mybir.dt.float32r
AF = mybir.ActivationFunctionType


def _build():
    nc = bacc.Bacc("TRN2", target_bir_lowering=False, debug=False,
                   num_devices=N_CORES)

    xT = nc.dram_tensor("xT", [C, N], F32R, kind="ExternalInput")
    w_qkv = nc.dram_tensor("w_qkv", [C, 3 * C], F32R, kind="ExternalInput")
    w_proj = nc.dram_tensor("w_proj", [C, C], F32R, kind="ExternalInput")
    b_qk = nc.dram_tensor("b_qk", [2 * KT, 128, 1], F32, kind="ExternalInput")
    b_v = nc.dram_tensor("b_v", [128, C], F32, kind="ExternalInput")
    b_o = nc.dram_tensor("b_o", [128, C], F32, kind="ExternalInput")
    ones12 = nc.dram_tensor("ones12", [128, H], F32R, kind="ExternalInput")
    y = nc.dram_tensor("y", [N, C], F32, kind="ExternalOutput")
    dbg = {}
    if DEBUG_TAPS:
        dbg["oa0"] = nc.dram_tensor("oa0", [D + 1, N], F32, kind="ExternalOutput")
        dbg["rz0"] = nc.dram_tensor("rz0", [1, N], F32, kind="ExternalOutput")
        dbg["rzb0"] = nc.dram_tensor("rzb0", [D, N], F32, kind="ExternalOutput")
        dbg["onorm0"] = nc.dram_tensor("onorm0", [128, N], F32, kind="ExternalOutput")
        dbg["pt00"] = nc.dram_tensor("pt00", [128, N], F32, kind="ExternalOutput")
        dbg["qk0"] = nc.dram_tensor("qk0", [128, N], F32, kind="ExternalOutput")
        dbg["kk0"] = nc.dram_tensor("kk0", [128, N], F32, kind="ExternalOutput")
        dbg["va0"] = nc.dram_tensor("va0", [128, H * (D + 1)], F32, kind="ExternalOutput")

    with tile.TileContext(nc) as tc:
        if BENCH_ITERS > 0:
            with tc.For_i(0, BENCH_ITERS, 1):
                _body(nc, tc, xT, w_qkv, w_proj, b_qk, b_v, b_o, ones12, y, dbg)
        else:
            _body(nc, tc, xT, w_qkv, w_proj, b_qk, b_v, b_o, ones12, y, dbg)
    nc.compile()
    return nc


def _body(nc, tc, xT, w_qkv, w_proj, b_qk, b_v, b_o, ones12, y, dbg={}):
    import contextlib
    ctx = contextlib.ExitStack()
    with ctx:
        # SBUF pools (bufs is per-tag)
        big = ctx.enter_context(tc.tile_pool(name="big", bufs=9))       # xT (6) + P^T churn (3)
        qk_pool = ctx.enter_context(tc.tile_pool(name="qk", bufs=8))    # rotating chunk tiles
        vaug_pool = ctx.enter_context(tc.tile_pool(name="vaug", bufs=1))
        onorm_pool = ctx.enter_context(tc.tile_pool(name="onorm", bufs=1))
        wqk_pool = ctx.enter_context(tc.tile_pool(name="wqk", bufs=18))
        wv_pool = ctx.enter_context(tc.tile_pool(name="wv", bufs=1))
        bias_pool = ctx.enter_context(tc.tile_pool(name="bias", bufs=1))
        ysb_pool = ctx.enter_context(tc.tile_pool(name="ysb", bufs=2))
        u_pool = ctx.enter_context(tc.tile_pool(name="u", bufs=2))
        zrow_pool = ctx.enter_context(tc.tile_pool(name="zrow", bufs=2))
        rz_pool = ctx.enter_context(tc.tile_pool(name="rz", bufs=2))
        rzb_pool = ctx.enter_context(tc.tile_pool(name="rzb", bufs=2))
        # PSUM: st/vc/yp churn (4 banks) + oa/pc accumulators (4 banks)
        ps_st = ctx.enter_context(tc.tile_pool(name="ps_st", bufs=2, space="PSUM"))
        ps_acc = ctx.enter_context(tc.tile_pool(name="ps_acc", bufs=2, space="PSUM"))

        qk_sb = {}
        wqk_tiles = {}
        bqk_sb = {}
        xt_sb = [None] * KT
        wv_sb = [None] * KT

        def stage_wqk(cc, eng):
            tiles = []
            for kt in range(KT):
                wt = wqk_pool.tile([128, 128], F32R, tag="wqk",
                                   name=f"wqk{cc}_{kt}")
                eng.dma_start(
                    wt[:], w_qkv.ap()[kt * 128:(kt + 1) * 128,
                                      cc * 128:(cc + 1) * 128])
                tiles.append(wt)
            wqk_tiles[cc] = tiles
            t = bias_pool.tile([128, 1], F32, tag=f"bqk{cc}", name=f"bqk{cc}")
            eng.dma_start(t[:], b_qk.ap()[cc])
            bqk_sb[cc] = t

        # ---- first-use-order staging across both HWDGE queues ----
        stage_wqk(0, nc.sync)
        stage_wqk(KT, nc.scalar)
        for kt in range(KT):
            t = big.tile([128, N], F32R, tag="big", name=f"xt{kt}")
            eng = nc.sync if kt < 4 else nc.gpsimd
            eng.dma_start(t[:, 0:512],
                          xT.ap()[kt * 128:(kt + 1) * 128, 0:512])
            eng.dma_start(t[:, 512:1024],
                          xT.ap()[kt * 128:(kt + 1) * 128, 512:1024])
            xt_sb[kt] = t
        for kt in range(KT):
            t = wv_pool.tile([128, C], F32R, tag=f"w2_{kt}", name=f"wv{kt}")
            nc.gpsimd.dma_start(
                t[:], w_qkv.ap()[kt * 128:(kt + 1) * 128, 2 * C:3 * C])
            wv_sb[kt] = t
        bv_sb = bias_pool.tile([128, C], F32, tag="bv")
        nc.gpsimd.dma_start(bv_sb[:], b_v.ap())
        ones_sb = bias_pool.tile([128, H], F32R, tag="ones")
        nc.gpsimd.dma_start(ones_sb[:], ones12.ap())

        # chunk production as resumable per-kt pieces (PE filler work)
        chunk_state = {}

        def chunk_piece(cc):
            kt = chunk_state.get(cc, 0)
            if kt >= KT:
                return
            if kt == 0:
                chunk_state[("pc", cc)] = ps_acc.tile(
                    [128, N], F32, tag="oa", name=f"pc{cc}")
            pc = chunk_state[("pc", cc)]
            wt = wqk_tiles[cc][kt]
            for half in range(2):
                s = slice(half * 512, (half + 1) * 512)
                nc.tensor.matmul(pc[:, s], wt[:], xt_sb[kt][:, s],
                                 start=(kt == 0), stop=(kt == KT - 1))
            chunk_state[cc] = kt + 1
            if kt == KT - 1:
                t = qk_pool.tile([128, N], F32R, tag="qk", name=f"qkc{cc}")
                nc.vector.tensor_scalar_add(t[:], pc[:], bqk_sb[cc][:])
                qk_sb[cc] = t
                del chunk_state[("pc", cc)]

        def chunk_mm(cc):
            for _ in range(KT):
                chunk_piece(cc)

        vaug_sb = [None] * MT

        def make_vaug(mt):
            vc = ps_st.tile([128, C], F32, tag="st", name=f"vc{mt}")
            for kt in range(KT):
                nc.tensor.matmul(vc[:, 0:512],
                                 xt_sb[kt][:, mt * 128:(mt + 1) * 128],
                                 wv_sb[kt][:, 0:512],
                                 start=(kt == 0), stop=(kt == KT - 1))
                nc.tensor.matmul(vc[:, 512:768],
                                 xt_sb[kt][:, mt * 128:(mt + 1) * 128],
                                 wv_sb[kt][:, 512:768],
                                 start=(kt == 0), stop=(kt == KT - 1))
            va = vaug_pool.tile([128, H * (D + 1)], F32R, tag=f"vaug{mt}",
                                name=f"vaug{mt}")
            va_h = va[:].rearrange("p (h s) -> p h s", h=H)
            nc.vector.tensor_copy(va_h[:, :, D], ones_sb[:])
            nc.vector.tensor_add(
                va_h[:, :, 0:D],
                vc[:].rearrange("p (h s) -> p h s", h=H),
                bv_sb[:].rearrange("p (h s) -> p h s", h=H))
            vaug_sb[mt] = va

        onorm_sb = [onorm_pool.tile([128, N], F32R, tag=f"onorm{i}",
                                    name=f"onorm{i}")
                    for i in range(KT)]

        class Head:
            def __init__(self, h):
                self.h = h
                self.qt = qk_sb[h // 2][(h % 2) * D:(h % 2) * D + D, :]
                self.kt = qk_sb[KT + h // 2][(h % 2) * D:(h % 2) * D + D, :]
                self.oa = ps_acc.tile([128, N], F32, tag="oa", name=f"oa{h}")
                self.pts = []

            def st_exp(self, mc):
                st = ps_st.tile([128, N], F32, tag="st",
                                name=f"st{self.h}_{mc}")
                for half in range(2):
                    s = slice(half * 512, (half + 1) * 512)
                    nc.tensor.matmul(st[:, s],
                                     self.kt[:, mc * 128:(mc + 1) * 128],
                                     self.qt[:, s], start=True, stop=True)
                pt = big.tile([128, N], F32R, tag="big",
                              name=f"pt{self.h}_{mc}")
                nc.scalar.activation(pt[:], st[:], AF.Exp,
                                     scale=float(D) ** -0.5)
                self.pts.append(pt)

            def pv(self, mc):
                h = self.h
                for half in range(2):
                    s = slice(half * 512, (half + 1) * 512)
                    nc.tensor.matmul(
                        self.oa[0:D + 1, s],
                        vaug_sb[mc][:, h * (D + 1):(h + 1) * (D + 1)],
                        self.pts[mc][:, s],
                        start=(mc == 0), stop=(mc == MT - 1))

            def norm(self):
                # stash oa in SBUF fast (frees the PSUM accumulator for the
                # next head), then normalize off the critical path
                h, oa = self.h, self.oa
                u = u_pool.tile([D, N], F32, tag="u", name=f"u{h}")
                nc.vector.tensor_copy(u[:], oa[0:D, :])
                zrow = zrow_pool.tile([1, N], F32, tag="zrow", name=f"z{h}")
                nc.vector.tensor_copy(zrow[:], oa[D:D + 1, :])
                rz = rz_pool.tile([1, N], F32, tag="rz", name=f"rz{h}")
                nc.vector.reciprocal_approx_fast(rz[:], zrow[:])
                rzb = rzb_pool.tile([D, N], F32, tag="rzb", name=f"rzb{h}")
                nc.gpsimd.partition_broadcast(rzb[:], rz[:])
                nc.vector.tensor_mul(
                    onorm_sb[h // 2][(h % 2) * D:(h % 2) * D + D, :],
                    u[:], rzb[:])
                if dbg and h == 0:
                    nc.sync.dma_start(dbg["oa0"].ap()[0:D, :], u[:])
                    nc.sync.dma_start(dbg["oa0"].ap()[D:D + 1, :], zrow[:])
                    nc.sync.dma_start(dbg["pt00"].ap(),
                                      self.pts[0][:].bitcast(F32))
                    nc.sync.dma_start(dbg["qk0"].ap(), qk_sb[0][:].bitcast(F32))
                    nc.sync.dma_start(dbg["kk0"].ap(),
                                      qk_sb[KT][:].bitcast(F32))
                    nc.sync.dma_start(dbg["va0"].ap(),
                                      vaug_sb[0][:].bitcast(F32))
                    nc.sync.dma_start(dbg["rz0"].ap(), rz[:])
                    nc.sync.dma_start(dbg["rzb0"].ap(), rzb[:])
                if dbg and h == 1:
                    nc.sync.dma_start(dbg["onorm0"].ap(),
                                      onorm_sb[0][:].bitcast(F32))

        def do_head(h, fillers=(), lookahead=2):
            # fillers: list of zero-arg callables emitting ~1 PE-matmul-pair
            hd = Head(h)
            fi = iter(fillers)
            for mc in range(MT):
                hd.st_exp(mc)
                for f in [next(fi, None)]:
                    if f:
                        f()
                if mc >= lookahead:
                    hd.pv(mc - lookahead)
            for f in fi:
                f()
            for mc in range(MT - lookahead, MT):
                hd.pv(mc)
            hd.norm()

        # ---- bootstrap: chunks 0/6, head 0 interleaved with V production ----
        chunk_mm(0)
        chunk_mm(KT)
        h0 = Head(0)
        for mc in range(MT):
            make_vaug(mc)
            h0.st_exp(mc)
            h0.pv(mc)
        h0.norm()

        # w_proj staging (reuses wv slots; lands after V consumed them)
        wp_sb = []
        for kt in range(KT):
            t = wv_pool.tile([128, C], F32R, tag=f"w2_{kt}", name=f"wp{kt}")
            nc.sync.dma_start(t[:], w_proj.ap()[kt * 128:(kt + 1) * 128, :])
            wp_sb.append(t)
        bo_sb = bias_pool.tile([128, C], F32, tag="bo")
        nc.sync.dma_start(bo_sb[:], b_o.ap())

        # head h produces the chunk needed one pair ahead:
        #   head 1 -> chunks 1 and KT+1; heads 2..9 -> chunk for pair h//2+1
        stage_wqk(1, nc.sync)
        stage_wqk(KT + 1, nc.sync)
        f1 = [lambda cc=1: chunk_piece(cc) for _ in range(KT)] + \
             [lambda cc=KT + 1: chunk_piece(cc) for _ in range(KT)]
        do_head(1, f1)
        for h in range(2, H):
            p = h // 2 + 1
            if p < KT:
                cc = p if h % 2 == 0 else KT + p
                stage_wqk(cc, nc.sync)
                fillers = [lambda c=cc: chunk_piece(c) for _ in range(KT)]
            else:
                fillers = []
            do_head(h, fillers)

        # ---- projection y[n, c] ----
        for nch in range(NCH):
            yp = ps_st.tile([128, C], F32, tag="st", name=f"yp{nch}")
            for kt in range(KT):
                nc.tensor.matmul(yp[:, 0:512],
                                 onorm_sb[kt][:, nch * 128:(nch + 1) * 128],
                                 wp_sb[kt][:, 0:512],
                                 start=(kt == 0), stop=(kt == KT - 1))
                nc.tensor.matmul(yp[:, 512:768],
                                 onorm_sb[kt][:, nch * 128:(nch + 1) * 128],
                                 wp_sb[kt][:, 512:768],
                                 start=(kt == 0), stop=(kt == KT - 1))
            ys = ysb_pool.tile([128, C], F32, tag="ysb", name=f"ys{nch}")
            nc.vector.tensor_add(ys[:], yp[:], bo_sb[:])
            nc.sync.dma_start(y.ap()[nch * 128:(nch + 1) * 128, :], ys[:])


_NC_CACHE = None


def _get_nc():
    global _NC_CACHE
    if _NC_CACHE is None:
        _NC_CACHE = _build()
    return _NC_CACHE


def make_in_maps(x, w_qkv, b_qkv, w_proj, b_proj):
    x = np.asarray(x, np.float32)
    w_qkv = np.ascontiguousarray(np.asarray(w_qkv, np.float32))
    b_qkv = np.asarray(b_qkv, np.float32)
    w_proj = np.ascontiguousarray(np.asarray(w_proj, np.float32))
    b_proj = np.asarray(b_proj, np.float32)

    b_qk = np.ascontiguousarray(b_qkv[:2 * C].reshape(2 * KT, 128, 1))
    b_v = np.ascontiguousarray(
        np.broadcast_to(b_qkv[2 * C:], (128, C)).astype(np.float32))
    b_o = np.ascontiguousarray(
        np.broadcast_to(b_proj, (128, C)).astype(np.float32))
    ones = np.ones((128, H), np.float32)

    in_maps = []
    for c in range(N_CORES):
        in_maps.append({
            "xT": np.ascontiguousarray(x[c].T),
            "w_qkv": w_qkv,
            "w_proj": w_proj,
            "b_qk": b_qk,
            "b_v": b_v,
            "b_o": b_o,
            "ones12": ones,
        })
    return in_maps


def kernel(x, w_qkv, b_qkv, w_proj, b_proj):
    nc = _get_nc()
    in_maps = make_in_maps(x, w_qkv, b_qkv, w_proj, b_proj)
    res = bass_utils.run_bass_kernel_spmd(nc, in_maps, list(range(N_CORES)))
    out = np.stack([res.results[c]["y"] for c in range(N_CORES)], axis=0)
    return out.astype(np.float32)
